# revision 19
# baseline (speedup 1.0000x reference)
"""Trainium2 Bass kernel for additive (tanh) attention with mask.

Computation (per batch b):
    wah    = h @ W_ah.T                             [B, H]
    e      = tanh(wah[:, None, :] + p_att_feats)    [B, M, H]
    logits = e @ w_alpha                            [B, M]
    logits = where(mask == 0, -1e9, logits)
    alpha  = softmax(logits, -1)
    att    = alpha @ att_feats                      [B, D]

Strategy: pure data-parallel over batch (8 batches / core on 8 cores).
Masked rows contribute exactly 0 to the softmax-weighted sum, so only
the ~50% of att_feats / p_att_feats rows with mask==1 ever reach the
device.  The mask is known at marshalling time, so the host compacts
those rows directly into SBUF layout order: all 8 slots packed
back-to-back at 16-row granularity (global row r = chunk*128 +
partition), f-stream [128, NCH, 2048] bf16 and p-stream
[128, NCH, 512] fp8-e3m4 (end-to-end rel-err 6e-3, gate 2e-2).  The
device streams them with plain contiguous HWDGE dma_start loads in
4-chunk pieces, alternating both HWDGE rings (qSPDynamicHW /
qActDynamicHW) -- dual-ring is worth ~20% measured over one ring;
per-row SWDGE dma_gather was descriptor-overhead bound at ~2x the
bandwidth deficit.

exp() is applied without max-subtraction (logits are bounded:
|logits| <= ||w_alpha||_1 with e in [-1,1]), masked/pad rows get an
additive -1e9 bias so their exp underflows to exactly 0, and the
normalization by 1/sum is applied once at PSUM drain time.  Slot
boundaries fall mid-chunk; DVE/ACT ops use exact partition subranges
while PE matmuls always contract from partition 0 (exr rows outside
the slot are memset-zero, contributing nothing) so the PE never sees a
nonzero partition offset.

Numerics vs fp32 reference: rel-err ~6e-3 (bf16 feats + fp8 p).
KERNEL_P_DTYPE=bf16 gives ~2.5e-3 at ~10% more DMA.

Implementation notes (hard-won):
  - InstTensorTensorReduce crashes the NRT exec on this runtime; the
    logits dot-product uses the fused scalar_tensor_tensor (+accum
    row-sum), which is fine on HW.
  - float32r matmul operands must be *produced* as float32r (BIR
    verifier); DRAM tensors are declared f32r/bf16 and exp() writes the
    PE-weight tile in that dtype directly.
  - Cross-partition reduction for the softmax denominator is a DVE-only
    copy/add log-tree + 32x32 stream transpose.
  - Phase-1 SBUF pools are opened before the phase-0 scratch pool so
    the stack allocator gives them non-overlapping addresses (otherwise
    a false overlap-dependency stalls the first loads).
  - wah row broadcast to 128 partitions uses a one-hot lhsT matmul
    (oh_j.T @ wah) -- no SBUF->SBUF DMA on the critical path.
  - Piece DMAs are issued interleaved with slot compute (lookahead 2)
    so no engine's in-order queue ever waits on a WAR semaphore whose
    releasing compute has not been emitted yet.

Self-contained: hardcodes B=64, M=1024, RNN=1024, H=512, D=2048, 8 cores.
"""

import os

import numpy as np

import concourse.bacc as bacc
import concourse.bass as bass
import concourse.mybir as mybir
from concourse import bass_isa, library_config
from concourse.bass_utils import run_bass_kernel_spmd
from concourse.tile import TileContext

B, M, RNN, H, D = 64, 1024, 1024, 512, 2048
NCORES = 8
BL = B // NCORES  # batches per core
NEG = -1e9
F32 = mybir.dt.float32
F32R = mybir.dt.float32r
BF16 = mybir.dt.bfloat16

# Dtype of the compacted att_feats stream + PE weighted-sum matmul:
#   bf16 (default): halves the dominant DMA stream; output err ~1e-3
#   f32r: full 4-byte stream, tf32-like matmul; output err ~2e-4
ATT_DT = os.environ.get("KERNEL_ATT_DTYPE", "bf16")
# Dtype of the compacted p_att_feats stream (tanh input):
#   f8e3 (default): fp8-e3m4, end-to-end rel-err ~6e-3
#   bf16: rel-err ~2.5e-3 at +10% DMA bytes
P_DT = os.environ.get("KERNEL_P_DTYPE", "f8e3")


def _plan(mask: np.ndarray):
    """Assign batches to (core, slot) balanced by unmasked count; compute
    per-slot padded sizes (identical across cores - SPMD)."""
    n = mask.sum(axis=1).astype(np.int64)  # [B]
    order = np.argsort(-n, kind="stable")
    batch_of = np.empty((NCORES, BL), dtype=np.int64)
    for j in range(BL):
        for c in range(NCORES):
            batch_of[c, j] = order[j * NCORES + c]
    nbar = np.empty(BL, dtype=np.int64)
    for j in range(BL):
        mx = max(int(n[batch_of[c, j]]) for c in range(NCORES))
        # 32-aligned so packed slot boundaries are legal DVE/ACT
        # partition offsets (HW requires offset % 32 == 0)
        nbar[j] = ((mx + 31) // 32) * 32
    nch = [(int(v) + 127) // 128 for v in nbar]
    return batch_of, n, nbar, nch


def _pparts(p0, p1):
    """Decompose [p0, p1) into HW-legal partition ranges: a range must
    fit a naturally-aligned power-of-2 block (count <= alignment of its
    start; start 0 has alignment 128)."""
    out = []
    while p0 < p1:
        a = 128 if p0 == 0 else (p0 & -p0)
        out.append((p0, min(p0 + a, p1)))
        p0 = min(p0 + a, p1)
    return out


def _geom(nbar, pc):
    """Packed-layout geometry: global row offsets, chunk count, pieces."""
    goff = np.cumsum([0] + [int(v) for v in nbar])
    R = int(goff[-1])
    NCH = (R + 127) // 128
    NP = (NCH + pc - 1) // pc
    return goff, R, NCH, NP


def _slotgeom(nbar):
    """Per-slot chunk spans and slot-local bias-column offsets."""
    goff = np.cumsum([0] + [int(v) for v in nbar])
    spans = []
    boffs = [0]
    for j in range(len(nbar)):
        g0, g1 = int(goff[j]), int(goff[j] + nbar[j])
        cs, ce = g0 // 128, (g1 - 1) // 128
        spans.append((cs, ce))
        boffs.append(boffs[-1] + (ce - cs + 1))
    return spans, boffs


def _build(nbar, nch, reps=1, bench_mode=False, loop_n=0, fsplit=2,
           ring=16384, fbufs=4, spkt=False, ring2=True, pc=4,
           p_gpsimd=False, f3way=False,
           ab_nocompute=False, ab_nomm=False, ab_nodma=False,
           ab_dmaonly=False):
    """Build the SPMD bass program (same for all cores).  reps>1 repeats
    phase 1 (benchmark amplification only; outputs are overwritten).
    bench_mode replaces the bulk inputs with device-side zero-filled
    internal DRAM so per-call host transfer is tiny."""
    goff, R, NCH, NP = _geom(nbar, pc)

    FATT = {"bf16": mybir.dt.bfloat16, "f32r": F32R}[ATT_DT]
    FP = {"f8e3": mybir.dt.float8e3, "f8e4": mybir.dt.float8e4,
          "bf16": mybir.dt.bfloat16}[P_DT]
    nc = bacc.Bacc("TRN2", target_bir_lowering=False)
    if bench_mode:
        f_d = nc.dram_tensor("feats_i", [128, NCH, D], FATT)
        p_d = nc.dram_tensor("pfeat_i", [128, NCH, H], FP)
    else:
        f_d = nc.dram_tensor("feats", [128, NCH, D], FATT, kind="ExternalInput")
        p_d = nc.dram_tensor("pfeat", [128, NCH, H], FP, kind="ExternalInput")
    # W^T and h^T arrive pre-permuted from the host (layout marshalling):
    # wt[p, rc, hh] = W[hh, rc*128+p], ht[p, rc, b] = h[b, rc*128+p].
    # f32r dram views let the PE consume them at 1 cycle/row.
    wt_d = nc.dram_tensor("wt", [128, RNN // 128, H], F32R, kind="ExternalInput")
    ht_d = nc.dram_tensor("ht", [128, RNN // 128, BL], F32R, kind="ExternalInput")
    wa_d = nc.dram_tensor("walpha", [1, H], F32R, kind="ExternalInput")
    # oh[b, j*128+p] = (b == j): one-hot lhsT used to broadcast row j of the
    # [BL, H] wah tile to all 128 partitions without any SBUF->SBUF move
    oh_d = nc.dram_tensor("oh", [BL, BL * 128], F32R, kind="ExternalInput")
    spans, boffs = _slotgeom(nbar)
    TB = boffs[-1]  # total slot-local bias columns
    bias_d = nc.dram_tensor("bias", [128, TB], F32, kind="ExternalInput")
    ones_d = nc.dram_tensor("ones", [1, 128], F32R, kind="ExternalInput")
    out_d = nc.dram_tensor("out", [BL, D], F32, kind="ExternalOutput")

    RC = RNN // 128  # 8

    with TileContext(nc) as tc:
        # Pool order matters: phase-1 pools are allocated BEFORE the
        # phase-0 scratch pool so their SBUF addresses do not overlap it.
        with (
            tc.tile_pool(name="const", bufs=1) as cp,
            tc.tile_pool(name="fp", bufs=fbufs) as fp,
            tc.tile_pool(name="pp", bufs=fbufs) as pp,
            tc.tile_pool(name="lp", bufs=4) as lp,
            tc.tile_pool(name="wk", bufs=4) as wk,
            tc.tile_pool(name="sm", bufs=3) as sm,
            tc.tile_pool(name="op", bufs=2) as op,
        ):
            if bench_mode:
                # zero-fill the internal bulk tensors once (phase -1)
                with tc.tile_pool(name="fill", bufs=1) as fillp:
                    ztf = fillp.tile([128, 1, D], FATT)
                    nc.vector.memset(ztf[:, :, :], 0.0)
                    ztp = fillp.tile([128, 1, H], FP)
                    nc.vector.memset(ztp[:, :, :], 0.0)
                    for c in range(NCH):
                        nc.sync.dma_start(f_d[:, c : c + 1, :], ztf[:, :, :])
                        nc.sync.dma_start(p_d[:, c : c + 1, :], ztp[:, :, :])
            bias_t = cp.tile([128, TB], F32)
            nc.sync.dma_start(bias_t[:, :], bias_d[:, :])
            wahb = cp.tile([128, BL, H], F32)  # per-slot wah broadcast
            walphab = cp.tile([128, H], F32)  # w_alpha broadcast

            # ---------------- phase 0: wah = h @ W.T, broadcasts ----------
            with (
                tc.tile_pool(name="ph0", bufs=1) as p0,
                tc.tile_pool(name="ph0ps", bufs=2, space="PSUM") as p0ps,
            ):
                ones_sb = p0.tile([1, 128], F32R)
                nc.sync.dma_start(ones_sb[:, :], ones_d[:, :])
                oh_sb = p0.tile([BL, BL * 128], F32R)
                nc.sync.dma_start(oh_sb[:, :], oh_d[:, :])
                wa_sb = p0.tile([1, H], F32R)
                nc.sync.dma_start(wa_sb[:, :], wa_d[:, :])
                wt_sb = p0.tile([128, RC, H], F32R)
                nc.sync.dma_start(wt_sb[:, :, :], wt_d[:, :, :])
                ht_sb = p0.tile([128, RC, BL], F32R)
                nc.sync.dma_start(ht_sb[:, :, :], ht_d[:, :, :])

                # wah [b, h] = sum_r h^T.T @ W^T
                ps_wah = p0ps.tile([BL, H], F32, tag="wah")
                for rc in range(RC):
                    nc.tensor.matmul(
                        ps_wah[:, :],
                        ht_sb[:, rc, :],
                        wt_sb[:, rc, :],
                        start=(rc == 0),
                        stop=(rc == RC - 1),
                    )
                wah_sb = p0.tile([BL, H], F32R)
                nc.vector.tensor_copy(wah_sb[:, :], ps_wah[:, :])
                # broadcast row j to 128 partitions: onehot_j.T @ wah_sb
                for j in range(BL):
                    pb = p0ps.tile([128, H], F32, tag="bc")
                    nc.tensor.matmul(
                        pb[:, :],
                        oh_sb[:, j * 128 : (j + 1) * 128],
                        wah_sb[:, :],
                        start=True, stop=True,
                    )
                    nc.scalar.copy(wahb[:, j, :], pb[:, :])
                pb = p0ps.tile([128, H], F32, tag="bc")
                nc.tensor.matmul(
                    pb[:, :], ones_sb[:, :], wa_sb[:, :], start=True, stop=True
                )
                nc.scalar.copy(walphab[:, :], pb[:, :])

            # ---------------- phase 1: packed sparse attention ------------
            import contextlib

            with tc.tile_pool(name="aps", bufs=2, space="PSUM") as aps:
                loop_cm = (
                    tc.For_i(0, loop_n, 1,
                             hint_engines=tuple(mybir.ALL_ENGINES))
                    if loop_n else contextlib.nullcontext()
                )
                with loop_cm:
                  for rep in range(reps):
                    pieces = {}
                    state = {"issued": 0}

                    if ab_nodma:
                        # compute-only ablation: read resident const tiles
                        p_cst = lp.tile([128, pc, H], FP, tag="pcst")
                        nc.vector.memset(p_cst[:, :, :], 0.25)
                        f_cst = lp.tile([128, pc, D], FATT, tag="fcst")
                        nc.vector.memset(f_cst[:, :, :], 0.25)

                    def issue_piece(pi):
                        if ab_nodma:
                            pieces[pi] = (p_cst, f_cst)
                            return
                        c0 = pi * pc
                        cw = min(pc, NCH - c0)
                        f_t = fp.tile([128, pc, D], FATT, tag="f")
                        p_t = pp.tile([128, pc, H], FP, tag="p")
                        # f-piece split across both HWDGE rings; p-piece
                        # alternates rings (or goes to SWDGE).
                        if f3way and cw >= 3:
                            # thirds across qSP / qAct / SWDGE queues
                            t = cw // 3
                            a = (cw - t + 1) // 2
                            b = cw - t - a
                            nc.sync.dma_start(
                                f_t[:, 0:a, :], f_d[:, c0 : c0 + a, :]
                            )
                            nc.scalar.dma_start(
                                f_t[:, a : a + b, :],
                                f_d[:, c0 + a : c0 + a + b, :],
                            )
                            nc.gpsimd.dma_start(
                                f_t[:, a + b : cw, :],
                                f_d[:, c0 + a + b : c0 + cw, :],
                            )
                        else:
                            half = (cw + 1) // 2
                            e1 = nc.scalar if ring2 else nc.sync
                            nc.sync.dma_start(
                                f_t[:, 0:half, :], f_d[:, c0 : c0 + half, :]
                            )
                            if cw > half:
                                e1.dma_start(
                                    f_t[:, half:cw, :],
                                    f_d[:, c0 + half : c0 + cw, :],
                                )
                        if p_gpsimd:
                            ep = nc.gpsimd
                        else:
                            ep = nc.scalar if (ring2 and pi % 2) else nc.sync
                        ep.dma_start(
                            p_t[:, 0:cw, :], p_d[:, c0 : c0 + cw, :]
                        )
                        if ab_dmaonly:
                            # minimal consumer to keep pool WAR pacing
                            cons = sm.tile([128, 1], F32, tag="cons")
                            nc.vector.tensor_reduce(
                                cons[:, :], f_t[:, cw - 1, 0:8],
                                axis=mybir.AxisListType.X,
                                op=mybir.AluOpType.add,
                            )
                            nc.vector.tensor_reduce(
                                cons[:, :], p_t[:, cw - 1, 0:8],
                                axis=mybir.AxisListType.X,
                                op=mybir.AluOpType.add,
                            )
                        pieces[pi] = (p_t, f_t)

                    def ensure(pi_target):
                        while state["issued"] < min(pi_target, NP):
                            issue_piece(state["issued"])
                            state["issued"] += 1

                    for j in range(BL):
                        g0, g1 = int(goff[j]), int(goff[j] + nbar[j])
                        cs, ce = g0 // 128, (g1 - 1) // 128
                        ncols = ce - cs + 1
                        ensure(ce // pc + 2)
                        if ab_dmaonly:
                            continue

                        logits = lp.tile([128, ncols], F32, tag=f"lg{ncols}")
                        nc.vector.memset(logits[:, :], 0.0)
                        exr = lp.tile([128, ncols], FATT, tag=f"ex{ncols}")
                        if ab_nocompute:
                            nc.vector.memset(exr[:, :], 0.0078125)
                        ps = None if ab_nomm else aps.tile([1, D], F32, tag="att")
                        for c in range(cs, ce + 1):
                            lc = c - cs
                            p0_ = g0 - c * 128 if c == cs else 0
                            p1_ = min(128, g1 - c * 128)
                            p_t, f_t = pieces[c // pc]
                            k = c % pc
                            e = wk.tile([128, H], BF16, tag="e")
                            tt = wk.tile([128, H], BF16, tag="tt")
                            for q0, q1 in ([] if ab_nocompute else _pparts(p0_, p1_)):
                                nc.vector.tensor_add(
                                    e[q0:q1, :], p_t[q0:q1, k, :],
                                    wahb[q0:q1, j, :],
                                )
                                nc.scalar.activation(
                                    e[q0:q1, :], e[q0:q1, :],
                                    mybir.ActivationFunctionType.Tanh,
                                )
                                # NOTE: InstTensorTensorReduce crashes the
                                # device (NRT exec error) on this runtime;
                                # the fused scalar_tensor_tensor (+accum
                                # row-sum) is fine.
                                nc.vector.scalar_tensor_tensor(
                                    out=tt[q0:q1, :],
                                    in0=e[q0:q1, :],
                                    scalar=1.0,
                                    in1=walphab[q0:q1, :],
                                    op0=mybir.AluOpType.mult,
                                    op1=mybir.AluOpType.mult,
                                    accum_out=logits[q0:q1, lc : lc + 1],
                                )
                        # exp(logits + bias) ONCE per slot; the per-slot
                        # bias table is -1e9 on pad rows AND on boundary
                        # rows belonging to neighbouring slots, so exr is
                        # exactly 0 outside this slot with no memset.
                        # Output dtype doubles as the PE weight dtype.
                        if not ab_nocompute:
                            lb = sm.tile([128, ncols], F32, tag=f"lb{ncols}")
                            nc.vector.tensor_add(
                                lb[:, :], logits[:, :],
                                bias_t[:, int(boffs[j]) : int(boffs[j]) + ncols],
                            )
                            nc.scalar.activation(
                                exr[:, :], lb[:, :],
                                mybir.ActivationFunctionType.Exp,
                            )
                        # PE runs the slot's matmuls back-to-back (p-state
                        # ramps to full clock).  Contracts from partition 0:
                        # rows outside the slot are exactly 0 in exr.
                        for c in ([] if ab_nomm else range(cs, ce + 1)):
                            lc = c - cs
                            p1_ = min(128, g1 - c * 128)
                            _, f_t = pieces[c // pc]
                            k = c % pc
                            lhsT = exr[0:p1_, lc : lc + 1]
                            for d in range(D // 512):
                                nc.tensor.matmul(
                                    ps[0:1, d * 512 : (d + 1) * 512],
                                    lhsT,
                                    f_t[0:p1_, k, d * 512 : (d + 1) * 512],
                                    start=(c == cs),
                                    stop=(c == ce),
                                )
                        # s = sum over all rows of exr (rows outside the slot
                        # are zero).  Partition reduction is a DVE-only
                        # log-tree (copy + add, then a 32x32 transpose).
                        rowsum = sm.tile([128, 1], F32, tag="rs")
                        nc.vector.tensor_reduce(
                            rowsum[:, :],
                            exr[:, :ncols],
                            axis=mybir.AxisListType.X,
                            op=mybir.AluOpType.add,
                        )
                        c1 = sm.tile([64, 1], F32, tag="c1")
                        nc.vector.tensor_copy(c1[:, :], rowsum[64:128, :])
                        a1 = sm.tile([64, 1], F32, tag="a1")
                        nc.vector.tensor_add(a1[:, :], rowsum[0:64, :], c1[:, :])
                        c2 = sm.tile([32, 1], F32, tag="c2")
                        nc.vector.tensor_copy(c2[:, :], a1[32:64, :])
                        stg = sm.tile([32, 32], F32, tag="stg")
                        nc.vector.memset(stg[:, :], 0.0)
                        nc.vector.tensor_add(stg[:, 0:1], a1[0:32, :], c2[:, :])
                        trp = sm.tile([32, 32], F32, tag="trp")
                        nc.vector.transpose(trp[:, :], stg[:, :])
                        sv = sm.tile([1, 1], F32, tag="sv")
                        nc.vector.tensor_reduce(
                            sv[0:1, :],
                            trp[0:1, :],
                            axis=mybir.AxisListType.X,
                            op=mybir.AluOpType.add,
                        )
                        rinv = sm.tile([1, 1], F32, tag="ri")
                        nc.vector.reciprocal(rinv[:, :], sv[:, :])
                        if not ab_nomm:
                            att = op.tile([1, D], F32, tag="at")
                            nc.scalar.activation(
                                att[:, :],
                                ps[0:1, :],
                                mybir.ActivationFunctionType.Copy,
                                scale=rinv[0:1, :],
                            )
                            nc.sync.dma_start(out_d[j : j + 1, :], att[:, :])
    nc.compile()
    return nc


_CACHE: dict = {}


def _get_compiled(mask: np.ndarray):
    key = mask.tobytes()
    hit = _CACHE.get("key") == key
    if not hit:
        batch_of, n, nbar, nch = _plan(mask)
        nc = _build(nbar, nch)
        _CACHE.update(
            key=key, nc=nc, batch_of=batch_of, n=n, nbar=nbar, nch=nch
        )
    return _CACHE


def kernel(h, att_feats, att_mask, p_att_feats, W_ah, w_alpha):
    h = np.ascontiguousarray(np.asarray(h, dtype=np.float32))
    att_feats = np.ascontiguousarray(np.asarray(att_feats, dtype=np.float32))
    mask = np.asarray(att_mask).astype(np.int32)
    p_att_feats = np.ascontiguousarray(np.asarray(p_att_feats, dtype=np.float32))
    W_ah = np.ascontiguousarray(np.asarray(W_ah, dtype=np.float32))
    w_alpha = np.ascontiguousarray(np.asarray(w_alpha, dtype=np.float32))

    st = _get_compiled(mask)
    nc, batch_of, n, nbar, nch = st["nc"], st["batch_of"], st["n"], st["nbar"], st["nch"]
    goff, R, NCH, NP = _geom(nbar, 4)

    import ml_dtypes

    feats_np = {"bf16": ml_dtypes.bfloat16, "f32r": np.float32}[ATT_DT]
    p_np = {"f8e3": ml_dtypes.float8_e3m4, "f8e4": ml_dtypes.float8_e4m3fn,
            "bf16": ml_dtypes.bfloat16}[P_DT]
    ones = np.ones((1, 128), dtype=np.float32)
    oh = np.zeros((BL, BL * 128), dtype=np.float32)
    for j in range(BL):
        oh[j, j * 128 : (j + 1) * 128] = 1.0
    wa_row = np.ascontiguousarray(w_alpha.reshape(1, H))
    # wt[p, rc, hh] = W_ah[hh, rc*128+p]
    wt_arr = np.ascontiguousarray(
        W_ah.T.reshape(RNN // 128, 128, H).transpose(1, 0, 2)
    )

    spans, boffs = _slotgeom(nbar)
    TB = boffs[-1]
    in_maps = []
    for c in range(NCORES):
        bids = batch_of[c]
        rows_f = np.zeros((NCH * 128, D), dtype=np.float32)
        rows_p = np.zeros((NCH * 128, H), dtype=np.float32)
        bias_arr = np.full((128, TB), NEG, dtype=np.float32)
        for j in range(BL):
            b = int(bids[j])
            nb = int(n[b])
            g0 = int(goff[j])
            rows = np.nonzero(mask[b])[0]
            assert rows.size == nb
            rows_f[g0 : g0 + nb] = att_feats[b][rows]
            rows_p[g0 : g0 + nb] = p_att_feats[b][rows]
            # per-slot bias columns: 0 only on this slot's valid rows
            cs, ce = spans[j]
            for lc in range(ce - cs + 1):
                r = (cs + lc) * 128 + np.arange(128)
                valid = (r >= g0) & (r < g0 + nb)
                bias_arr[valid, boffs[j] + lc] = 0.0
        # SBUF layout: global row r -> partition r%128, chunk r//128
        f_arr = np.ascontiguousarray(
            rows_f.reshape(NCH, 128, D).transpose(1, 0, 2)
        ).astype(feats_np)
        p_arr = np.ascontiguousarray(
            rows_p.reshape(NCH, 128, H).transpose(1, 0, 2)
        ).astype(p_np)
        bias_arr = np.ascontiguousarray(bias_arr)
        h_l = h[bids]  # [BL, RNN]
        ht_arr = np.ascontiguousarray(
            h_l.T.reshape(RNN // 128, 128, BL).transpose(1, 0, 2)
        )
        in_maps.append(
            {
                "feats": f_arr,
                "pfeat": p_arr,
                "wt": wt_arr,
                "ht": ht_arr,
                "walpha": wa_row,
                "bias": bias_arr,
                "ones": ones,
                "oh": oh,
            }
        )

    res = run_bass_kernel_spmd(nc, in_maps, core_ids=list(range(NCORES)))
    kernel._last_results = res  # for test harness introspection

    out = np.empty((B, D), dtype=np.float32)
    for c in range(NCORES):
        o = res.results[c]["out"]
        for j in range(BL):
            out[int(batch_of[c, j])] = o[j]
    return out
